# revision 17
# baseline (speedup 1.0000x reference)
"""Trainium2 Bass kernel for nn_AttentionLayer (B=2, S=2048, D=1024, H=16, dh=64).

Sharding: head-parallel across 8 NeuronCores — each core computes the Q/K/V
projections for its 2 heads (column slices of Wq/Wk/Wv), causal attention for
its 4 (batch, head) pairs, then an AllToAll exchanges per-head context so each
core runs the output projection for 1/8 of the tokens.

All matmuls run in bf16 (full PE rate + fast weight load); PSUM accumulation
stays fp32. Softmax skips the max-subtraction (|scores| <= 8 after the
1/sqrt(64) scale, since q/k are tanh outputs), so exp is a single ACT pass and
row sums come from an appended ones-column in the alpha @ V matmul.

Causal structure is exploited at 128-column granularity: for diagonal key
tiles the score/context matmuls cover only the valid query columns, and the
mask add is a single 128-wide triangular matmul.

The AllToAll is split per local head: the h=0 exchange overlaps the h=1
attention compute, and the output projection accumulates each 64-row half as
soon as its exchange lands (K=64 row-packed matmuls).

Self-contained: accepts the full unsharded inputs, returns the full output.
"""

import os

import numpy as np

import concourse.bass as bass
import concourse.mybir as mybir
import concourse.tile as tile
from concourse import bacc
from concourse.bass_utils import run_bass_kernel_spmd

B, S, D = 2, 2048, 1024
H, DH = 16, 64
N_CORES = 8
HPC = H // N_CORES          # heads per core (2)
LC = HPC * DH               # local projection columns (128)
T = B * S                   # total tokens (4096)
TBLK = T // N_CORES         # tokens per output block (512)
NEG = -1.0e9

f32 = mybir.dt.float32
f32r = mybir.dt.float32r
bf16 = mybir.dt.bfloat16

_CACHE = {}
LAST_RESULTS = None


def _build():
    nc = bacc.Bacc("TRN2", target_bir_lowering=False, debug=False,
                   num_devices=N_CORES)

    statesT = nc.dram_tensor("statesT", [D, T], bf16, kind="ExternalInput")
    wq = nc.dram_tensor("wq", [D, LC], bf16, kind="ExternalInput")
    wk = nc.dram_tensor("wk", [D, LC], bf16, kind="ExternalInput")
    wv = nc.dram_tensor("wv", [D, LC], bf16, kind="ExternalInput")
    wo = nc.dram_tensor("wo", [D, D], bf16, kind="ExternalInput")
    bq = nc.dram_tensor("bq", [LC, 1], f32, kind="ExternalInput")
    bk = nc.dram_tensor("bk", [LC, 1], f32, kind="ExternalInput")
    bv = nc.dram_tensor("bv", [LC, 1], f32, kind="ExternalInput")
    bo = nc.dram_tensor("bo", [D, 1], f32, kind="ExternalInput")
    tri = nc.dram_tensor("tri", [128, 128], bf16, kind="ExternalInput")
    ident16 = nc.dram_tensor("ident16", [128, 128], bf16, kind="ExternalInput")
    ones = nc.dram_tensor("ones", [128, 64], bf16, kind="ExternalInput")

    a2a_in = [nc.dram_tensor(f"a2a_in{h}", [N_CORES, DH, TBLK], bf16)
              for h in range(HPC)]
    a2a_out = [nc.dram_tensor(f"a2a_out{h}", [N_CORES, DH, TBLK], bf16)
               for h in range(HPC)]
    out = nc.dram_tensor("out", [D, TBLK], f32, kind="ExternalOutput")

    Tanh = mybir.ActivationFunctionType.Tanh
    Exp = mybir.ActivationFunctionType.Exp

    with tile.TileContext(nc) as tc:
        with (
            tc.tile_pool(name="consts", bufs=1) as consts,
            tc.tile_pool(name="persist", bufs=1) as persist,
            tc.tile_pool(name="stream", bufs=4) as stream,
            tc.tile_pool(name="vtp", bufs=2) as vtp,
            tc.tile_pool(name="etp", bufs=7) as etp,
            tc.tile_pool(name="cxp", bufs=2) as cxp,
            tc.tile_pool(name="outp", bufs=3) as outp,
            # one PSUM pool for the whole program: tag "mm" [128,2,512] x3 =
            # 6 banks, tag "acc" [128,512] x2 = 2 banks -> 8 banks total
            tc.tile_pool(name="psum", bufs=1, space="PSUM") as psum,
        ):
            # ---- constants / weights in SBUF ----
            # DMA order is startup-critical: biases + small consts first,
            # then weight slices interleaved with the first statesT tiles so
            # the first matmul can start as early as possible.
            bq_sb = consts.tile([LC, 1], f32)
            bk_sb = consts.tile([LC, 1], f32)
            bv_sb = consts.tile([LC, 1], f32)
            nc.sync.dma_start(out=bq_sb, in_=bq[:, :])
            nc.sync.dma_start(out=bk_sb, in_=bk[:, :])
            nc.sync.dma_start(out=bv_sb, in_=bv[:, :])
            ident_sb = consts.tile([128, 128], bf16)
            tri_sb = consts.tile([128, 128], bf16)
            ones_sb = consts.tile([128, 64], bf16)

            wq_sb = consts.tile([128, 8, LC], bf16)
            wk_sb = consts.tile([128, 8, LC], bf16)
            wv_sb = consts.tile([128, 8, LC], bf16)
            wq_ap = wq.ap().rearrange("(k p) l -> p k l", p=128)
            wk_ap = wk.ap().rearrange("(k p) l -> p k l", p=128)
            wv_ap = wv.ap().rearrange("(k p) l -> p k l", p=128)
            # wo/bo are only needed by phase 3; keep them on the gpsimd
            # queue behind the statesT tiles it also carries.
            wo_sb = persist.tile([128, 8, D], bf16)
            bo_sb = consts.tile([128, 8, 1], f32)

            # ---- phase 1: Q/K/V projections (transposed layout) ----
            # h-major tiles: partitions = (head*64 + d), free = (batch*S + s).
            # One tanh per projection per token tile, and the two batches'
            # score matmuls row-pack into different PE row groups via the
            # stationary partition offset.
            qt_sb = persist.tile([128, T], bf16, name="qt")
            kt_sb = persist.tile([128, T], bf16, name="kt")
            # v5: per 128-token tile, [tok_local, (h0 V | ones | h1 V | ones)]
            v5_sb = persist.tile([128, T // 128, 2 * (DH + 1)], bf16)

            for tt in range(T // 1024):  # 4 double-width token tiles
                # three passes (q, k, v) over prefetched statesT tiles: each
                # pass's PSUM slot was drained by a tanh two passes ago, so
                # the PE never waits at token-tile boundaries
                sts = []
                for kk in range(8):
                    if tt == 0:
                        nc.sync.dma_start(out=wq_sb[:, kk, :], in_=wq_ap[:, kk, :])
                        nc.sync.dma_start(out=wk_sb[:, kk, :], in_=wk_ap[:, kk, :])
                        nc.sync.dma_start(out=wv_sb[:, kk, :], in_=wv_ap[:, kk, :])
                    st = stream.tile([128, 1024], bf16, tag="st", bufs=12)
                    dma_eng = nc.sync if (tt < 2 or kk % 2 == 0) else nc.gpsimd
                    # per-half DMAs: the half-0 matmul starts as soon as its
                    # 128 KB lands (subtile deps), halving the startup wait
                    for half in range(2):
                        dma_eng.dma_start(
                            out=st[:, 512 * half:512 * (half + 1)],
                            in_=statesT[128 * kk:128 * (kk + 1),
                                        1024 * tt + 512 * half:
                                        1024 * tt + 512 * (half + 1)],
                        )
                    sts.append(st)
                    if tt == 0 and kk == 0:
                        # small consts off the first-matmul critical DMA path
                        nc.sync.dma_start(out=ident_sb, in_=ident16[:, :])
                        nc.sync.dma_start(out=tri_sb, in_=tri[:, :])
                        nc.sync.dma_start(out=ones_sb, in_=ones[:, :])
                        nc.vector.tensor_copy(
                            v5_sb.rearrange("p t (a b) -> p (t a) b", a=2)[:, :, DH:DH + 1].opt(),
                            ones_sb[:, :].opt(),
                        )
                bb = tt // 2                     # which batch this tt is in
                sl = slice(2048 * bb + 1024 * (tt % 2),
                           2048 * bb + 1024 * (tt % 2) + 1024)
                vt_c = vtp.tile([128, 1024], bf16, tag="vt")
                for w_sb, b_sb, out_ap in (
                        (wq_sb, bq_sb, qt_sb[:, sl]),
                        (wk_sb, bk_sb, kt_sb[:, sl]),
                        (wv_sb, bv_sb, vt_c[:, :])):
                    acc = psum.tile([128, 2, 512], f32, tag="mm", bufs=3)
                    for kk in range(8):
                        for half in range(2):
                            nc.tensor.matmul(acc[:, half, :], w_sb[:, kk, :],
                                             sts[kk][:, 512 * half:512 * (half + 1)],
                                             start=(kk == 0), stop=(kk == 7))
                    nc.scalar.activation(out=out_ap, in_=acc,
                                         func=Tanh, bias=b_sb)
                # transpose each 128-col block of vt into v5 (both heads at once)
                for j in range(8):
                    t_idx = 8 * tt + j
                    trp = psum.tile([128, 1024], bf16, tag="acc", bufs=2)
                    nc.tensor.transpose(trp[:, 0:128],
                                        vt_c[:, 128 * j:128 * (j + 1)], ident_sb)
                    nc.vector.tensor_copy(
                        v5_sb.rearrange("p t (a b) -> p t a b", a=2)[:, t_idx, :, 0:DH],
                        trp[:, 0:128].rearrange("p (a b) -> p a b", a=2),
                    )

            # ---- phase 2: causal attention, h-outer for split A2A ----
            # The inner loop is software-pipelined: score matmuls run two
            # groups ahead of the context matmuls so the PE never waits on
            # the exp, and each qi's normalize/flush chain is deferred into
            # the next qi's score stream so the PE broadcast matmul never
            # waits on the DVE reciprocal.
            for h in range(HPC):
                hp = DH * h

                def flush_group(grp):
                    # batched 1/l: copy each group's l-row to a distinct
                    # 32-aligned partition, one reciprocal serves them all
                    lb = cxp.tile([64, 512], f32, tag="lb", bufs=1)
                    for i, (cl_sb, _) in enumerate(grp):
                        nc.vector.tensor_copy(lb[32 * i:32 * i + 1, :],
                                              cl_sb[DH:DH + 1, :])
                    rbf = cxp.tile([64, 512], f32, tag="rbf", bufs=1)
                    nc.vector.reciprocal_approx_fast(out=rbf, in_=lb)
                    rbat = cxp.tile([64, 512], bf16, tag="rbat", bufs=1)
                    nc.vector.tensor_copy(rbat, rbf)
                    for i, (cl_sb, tb_idx) in enumerate(grp):
                        rbp = psum.tile([128, 512], f32, tag="acc", bufs=2)
                        nc.tensor.matmul(rbp[0:DH, :], ones_sb[32 * i:32 * i + 1, :],
                                         rbat[32 * i:32 * i + 1, :],
                                         start=True, stop=True,
                                         tile_position=(32 * i, 0))
                        cx = cxp.tile([DH, 512], bf16, tag="cx")
                        nc.vector.tensor_mul(cx, cl_sb[0:DH, :], rbp[0:DH, :])
                        nc.sync.dma_start(out=a2a_in[h][tb_idx, :, :], in_=cx)

                deferred = None  # previous qi's blocks awaiting normalize
                for qi in range(4):
                    nkt = 4 * qi + 4       # causal kt tiles (128 wide)
                    q_lo = 512 * qi
                    ctxps = [psum.tile([128, 512], f32, tag="acc", bufs=2,
                                       name=f"ctxp_h{h}q{qi}b{b}")
                             for b in range(B)]

                    def issue_ctx(ch, b, et):
                        for j in range(2):
                            ktj = 2 * ch + j
                            m = ktj - 4 * qi
                            lo = 128 * m if m > 0 else 0
                            nc.tensor.matmul(
                                ctxps[b][0:DH + 1, lo:512],
                                v5_sb[:, 16 * b + ktj, 65 * h:65 * h + 65],
                                et[:, j, lo:512],
                                start=(ktj == 0), stop=(ktj == nkt - 1),
                            )

                    # supergroups of 3: issue 3 groups' scores back-to-back,
                    # then their 3 ctx pairs — the exp runs 3 groups deep so
                    # the PE stream never waits on the ACT engine
                    gseq = [(c, b) for c in range(nkt // 2) for b in range(B)]
                    for ci in range(0, len(gseq), 3):
                        ets = []
                        for ch, b in gseq[ci:ci + 3]:
                            stp = psum.tile([128, 2, 512], f32, tag="mm", bufs=3)
                            for j in range(2):
                                ktj = 2 * ch + j
                                k_lo = 128 * ktj
                                m = ktj - 4 * qi  # >=0: diagonal key tile
                                lo = 128 * m if m > 0 else 0
                                nc.tensor.matmul(
                                    stp[:, j, lo:512],
                                    kt_sb[hp:hp + DH,
                                          2048 * b + k_lo:2048 * b + k_lo + 128],
                                    qt_sb[hp:hp + DH,
                                          2048 * b + q_lo + lo:2048 * b + q_lo + 512],
                                    start=True, stop=m < 0,
                                )
                                if m >= 0:  # causal bias via PE accumulate
                                    nc.tensor.matmul(
                                        stp[:, j, lo:lo + 128], ident_sb,
                                        tri_sb,
                                        start=False, stop=True,
                                    )
                            m0 = 2 * ch - 4 * qi  # j=0 diagonal index
                            elo = 128 * m0 if m0 > 0 else 0
                            et = etp.tile([128, 2, 512], bf16, tag="et")
                            nc.scalar.activation(out=et[:, :, elo:512],
                                                 in_=stp[:, :, elo:512],
                                                 func=Exp, scale=0.125)
                            ets.append((ch, b, et))
                        for ch, b, et in ets:
                            issue_ctx(ch, b, et)
                        if ci == 0 and deferred is not None:
                            flush_group(deferred)
                            deferred = None

                    group = []
                    for b in range(B):
                        # copy ctx+l out of PSUM eagerly (PSUM slot recycles)
                        cl_sb = cxp.tile([DH + 1, 512], f32, tag="cl", bufs=5)
                        nc.vector.tensor_copy(cl_sb, ctxps[b][0:DH + 1, :])
                        group.append((cl_sb, 4 * b + qi))
                    if qi == 3:            # last qi gates the A2A: no defer
                        flush_group(group)
                    else:
                        deferred = group
                # per-head exchange: h=0 overlaps h=1 compute
                nc.gpsimd.collective_compute(
                    "AllToAll", mybir.AluOpType.bypass,
                    replica_groups=[list(range(N_CORES))],
                    ins=[a2a_in[h][:].opt()], outs=[a2a_out[h][:].opt()],
                )

            # ---- phase 3: output projection, per-half accumulation ----
            nc.gpsimd.dma_start(out=wo_sb,
                                in_=wo.ap().rearrange("(k p) o -> p k o", p=128))
            nc.gpsimd.dma_start(out=bo_sb,
                                in_=bo.ap().rearrange("(k p) one -> p k one", p=128))
            cxt0s, cxt1s = [], []
            for kc in range(8):
                cxt0 = outp.tile([128, 512], bf16, tag="cxt0", bufs=8)
                cxt1 = outp.tile([128, 512], bf16, tag="cxt1", bufs=8)
                eng0 = nc.sync if kc % 2 == 0 else nc.gpsimd
                eng1 = nc.gpsimd if kc % 2 == 0 else nc.sync
                eng0.dma_start(out=cxt0[0:DH, :], in_=a2a_out[0][kc, :, :])
                eng1.dma_start(out=cxt1[DH:128, :], in_=a2a_out[1][kc, :, :])
                cxt0s.append(cxt0)
                cxt1s.append(cxt1)
            s0s = []
            for oc in range(8):  # h=0 half: runs as soon as A2A#1 lands
                op0 = psum.tile([128, 512], f32, tag="acc", bufs=2)
                osl = slice(128 * oc, 128 * (oc + 1))
                for kc in range(8):
                    nc.tensor.matmul(op0, wo_sb[0:DH, kc, osl], cxt0s[kc][0:DH, :],
                                     start=(kc == 0), stop=(kc == 7))
                s0 = outp.tile([128, 512], f32, tag="s0", bufs=8)
                nc.vector.tensor_copy(s0, op0)
                s0s.append(s0)
            for oc in range(8):  # h=1 half after A2A#2, then combine
                op1 = psum.tile([128, 512], f32, tag="acc", bufs=2)
                osl = slice(128 * oc, 128 * (oc + 1))
                for kc in range(8):
                    nc.tensor.matmul(op1, wo_sb[DH:128, kc, osl],
                                     cxt1s[kc][DH:128, :],
                                     start=(kc == 0), stop=(kc == 7))
                s1 = outp.tile([128, 512], f32, tag="s1", bufs=2)
                nc.vector.tensor_add(s1, s0s[oc], op1)
                osb = outp.tile([128, 512], f32, tag="osb", bufs=2)
                nc.scalar.activation(out=osb, in_=s1, func=Tanh, bias=bo_sb[:, oc, :])
                nc.sync.dma_start(out=out[osl, :], in_=osb)

    nc.compile()
    return nc


def _get_nc():
    if "nc" not in _CACHE:
        _CACHE["nc"] = _build()
    return _CACHE["nc"]


def kernel(states, Wq, bq, Wk, bk, Wv, bv, Wo, bo):
    global LAST_RESULTS
    import ml_dtypes
    bf = ml_dtypes.bfloat16
    states = np.asarray(states, dtype=np.float32)
    Wq, Wk, Wv, Wo = (np.asarray(w, dtype=np.float32) for w in (Wq, Wk, Wv, Wo))
    bq, bk, bv, bo = (np.asarray(x, dtype=np.float32) for x in (bq, bk, bv, bo))

    statesT = np.ascontiguousarray(states.reshape(T, D).T).astype(bf)
    # tri[k, c] = NEG where query column c (within the diagonal 128-block)
    # is strictly left of key row k
    k_idx = np.arange(128)[:, None]
    c_idx = np.arange(128)[None, :]
    tri = np.where(c_idx >= k_idx, 0.0, NEG).astype(bf)
    ident16 = np.eye(128, dtype=np.float32).astype(bf)
    ones = np.ones((128, 64), dtype=np.float32).astype(bf)

    in_maps = []
    for c in range(N_CORES):
        sl = slice(LC * c, LC * (c + 1))
        in_maps.append({
            "statesT": statesT,
            "wq": np.ascontiguousarray(Wq[:, sl]).astype(bf),
            "wk": np.ascontiguousarray(Wk[:, sl]).astype(bf),
            "wv": np.ascontiguousarray(Wv[:, sl]).astype(bf),
            "wo": Wo.astype(bf),
            "bq": np.ascontiguousarray(bq[sl]).reshape(LC, 1),
            "bk": np.ascontiguousarray(bk[sl]).reshape(LC, 1),
            "bv": np.ascontiguousarray(bv[sl]).reshape(LC, 1),
            "bo": bo.reshape(D, 1),
            "tri": tri,
            "ident16": ident16,
            "ones": ones,
        })

    nc = _get_nc()
    res = run_bass_kernel_spmd(nc, in_maps, core_ids=list(range(N_CORES)))
    LAST_RESULTS = res

    full = np.empty((T, D), dtype=np.float32)
    for c in range(N_CORES):
        full[TBLK * c:TBLK * (c + 1), :] = res.results[c]["out"].T
    return full.reshape(B, S, D)


# revision 20
# speedup vs baseline: 1.0062x; 1.0062x over previous
"""Trainium2 Bass kernel for nn_AttentionLayer (B=2, S=2048, D=1024, H=16, dh=64).

Sharding: head-parallel across 8 NeuronCores — each core computes the Q/K/V
projections for its 2 heads (column slices of Wq/Wk/Wv), causal attention for
its 4 (batch, head) pairs, then an AllToAll exchanges per-head context so each
core runs the output projection for 1/8 of the tokens.

All matmuls run in bf16 (full PE rate + fast weight load); PSUM accumulation
stays fp32. Softmax skips the max-subtraction (|scores| <= 8 after the
1/sqrt(64) scale, since q/k are tanh outputs), so exp is a single ACT pass and
row sums come from an appended ones-column in the alpha @ V matmul.

Causal structure is exploited at 128-column granularity: for diagonal key
tiles the score/context matmuls cover only the valid query columns, and the
mask add is a single 128-wide triangular matmul.

The AllToAll is split per local head: the h=0 exchange overlaps the h=1
attention compute, and the output projection accumulates each 64-row half as
soon as its exchange lands (K=64 row-packed matmuls).

Self-contained: accepts the full unsharded inputs, returns the full output.
"""

import os

import numpy as np

import concourse.bass as bass
import concourse.mybir as mybir
import concourse.tile as tile
from concourse import bacc
from concourse.bass_utils import run_bass_kernel_spmd

B, S, D = 2, 2048, 1024
H, DH = 16, 64
N_CORES = 8
HPC = H // N_CORES          # heads per core (2)
LC = HPC * DH               # local projection columns (128)
T = B * S                   # total tokens (4096)
TBLK = T // N_CORES         # tokens per output block (512)
NEG = -1.0e9

f32 = mybir.dt.float32
f32r = mybir.dt.float32r
bf16 = mybir.dt.bfloat16

_CACHE = {}
LAST_RESULTS = None


def _build():
    nc = bacc.Bacc("TRN2", target_bir_lowering=False, debug=False,
                   num_devices=N_CORES)

    statesT = nc.dram_tensor("statesT", [D, T], bf16, kind="ExternalInput")
    wq = nc.dram_tensor("wq", [D, LC], bf16, kind="ExternalInput")
    wk = nc.dram_tensor("wk", [D, LC], bf16, kind="ExternalInput")
    wv = nc.dram_tensor("wv", [D, LC], bf16, kind="ExternalInput")
    wo = nc.dram_tensor("wo", [D, D], bf16, kind="ExternalInput")
    bq = nc.dram_tensor("bq", [LC, 1], f32, kind="ExternalInput")
    bk = nc.dram_tensor("bk", [LC, 1], f32, kind="ExternalInput")
    bv = nc.dram_tensor("bv", [LC, 1], f32, kind="ExternalInput")
    bo = nc.dram_tensor("bo", [D, 1], f32, kind="ExternalInput")
    tri = nc.dram_tensor("tri", [128, 128], bf16, kind="ExternalInput")
    ident16 = nc.dram_tensor("ident16", [128, 128], bf16, kind="ExternalInput")
    ones = nc.dram_tensor("ones", [128, 64], bf16, kind="ExternalInput")

    a2a_in = [nc.dram_tensor(f"a2a_in{h}", [N_CORES, DH, TBLK], bf16)
              for h in range(HPC)]
    a2a_out = [nc.dram_tensor(f"a2a_out{h}", [N_CORES, DH, TBLK], bf16)
               for h in range(HPC)]
    out = nc.dram_tensor("out", [D, TBLK], f32, kind="ExternalOutput")

    Tanh = mybir.ActivationFunctionType.Tanh
    Exp = mybir.ActivationFunctionType.Exp

    with tile.TileContext(nc) as tc:
        with (
            tc.tile_pool(name="consts", bufs=1) as consts,
            tc.tile_pool(name="persist", bufs=1) as persist,
            tc.tile_pool(name="stream", bufs=4) as stream,
            tc.tile_pool(name="vtp", bufs=2) as vtp,
            tc.tile_pool(name="etp", bufs=7) as etp,
            tc.tile_pool(name="cxp", bufs=2) as cxp,
            tc.tile_pool(name="outp", bufs=3) as outp,
            # one PSUM pool for the whole program: tag "mm" [128,2,512] x3 =
            # 6 banks, tag "acc" [128,512] x2 = 2 banks -> 8 banks total
            tc.tile_pool(name="psum", bufs=1, space="PSUM") as psum,
        ):
            # ---- constants / weights in SBUF ----
            # DMA order is startup-critical: biases + small consts first,
            # then weight slices interleaved with the first statesT tiles so
            # the first matmul can start as early as possible.
            bq_sb = consts.tile([LC, 1], f32)
            bk_sb = consts.tile([LC, 1], f32)
            bv_sb = consts.tile([LC, 1], f32)
            nc.sync.dma_start(out=bq_sb, in_=bq[:, :])
            nc.sync.dma_start(out=bk_sb, in_=bk[:, :])
            nc.sync.dma_start(out=bv_sb, in_=bv[:, :])
            ident_sb = consts.tile([128, 128], bf16)
            tri_sb = consts.tile([128, 128], bf16)
            ones_sb = consts.tile([128, 64], bf16)

            wq_sb = consts.tile([128, 8, LC], bf16)
            wk_sb = consts.tile([128, 8, LC], bf16)
            wv_sb = consts.tile([128, 8, LC], bf16)
            wq_ap = wq.ap().rearrange("(k p) l -> p k l", p=128)
            wk_ap = wk.ap().rearrange("(k p) l -> p k l", p=128)
            wv_ap = wv.ap().rearrange("(k p) l -> p k l", p=128)
            # wo/bo are only needed by phase 3; keep them on the gpsimd
            # queue behind the statesT tiles it also carries.
            wo_sb = persist.tile([128, 8, D], bf16)
            bo_sb = consts.tile([128, 8, 1], f32)

            # ---- phase 1: Q/K/V projections (transposed layout) ----
            # h-major tiles: partitions = (head*64 + d), free = (batch*S + s).
            # One tanh per projection per token tile, and the two batches'
            # score matmuls row-pack into different PE row groups via the
            # stationary partition offset.
            qt_sb = persist.tile([128, T], bf16, name="qt")
            kt_sb = persist.tile([128, T], bf16, name="kt")
            # v5: per 128-token tile, [tok_local, (h0 V | ones | h1 V | ones)]
            v5_sb = persist.tile([128, T // 128, 2 * (DH + 1)], bf16)

            for tt in range(T // 1024):  # 4 double-width token tiles
                # three passes (q, k, v) over prefetched statesT tiles: each
                # pass's PSUM slot was drained by a tanh two passes ago, so
                # the PE never waits at token-tile boundaries
                sts = []
                for kk in range(8):
                    if tt == 0:
                        nc.sync.dma_start(out=wq_sb[:, kk, :], in_=wq_ap[:, kk, :])
                        nc.sync.dma_start(out=wk_sb[:, kk, :], in_=wk_ap[:, kk, :])
                        nc.sync.dma_start(out=wv_sb[:, kk, :], in_=wv_ap[:, kk, :])
                    st = stream.tile([128, 1024], bf16, tag="st", bufs=12)
                    dma_eng = nc.sync if (tt < 2 or kk % 2 == 0) else nc.gpsimd
                    dma_eng.dma_start(
                        out=st,
                        in_=statesT[128 * kk:128 * (kk + 1),
                                    1024 * tt:1024 * (tt + 1)],
                    )
                    sts.append(st)
                    if tt == 0 and kk == 0:
                        # small consts off the first-matmul critical DMA path
                        nc.sync.dma_start(out=ident_sb, in_=ident16[:, :])
                        nc.sync.dma_start(out=tri_sb, in_=tri[:, :])
                        nc.sync.dma_start(out=ones_sb, in_=ones[:, :])
                        nc.vector.tensor_copy(
                            v5_sb.rearrange("p t (a b) -> p (t a) b", a=2)[:, :, DH:DH + 1].opt(),
                            ones_sb[:, :].opt(),
                        )
                bb = tt // 2                     # which batch this tt is in
                sl = slice(2048 * bb + 1024 * (tt % 2),
                           2048 * bb + 1024 * (tt % 2) + 1024)
                vt_c = vtp.tile([128, 1024], bf16, tag="vt")
                for w_sb, b_sb, out_ap in (
                        (wq_sb, bq_sb, qt_sb[:, sl]),
                        (wk_sb, bk_sb, kt_sb[:, sl]),
                        (wv_sb, bv_sb, vt_c[:, :])):
                    acc = psum.tile([128, 2, 512], f32, tag="mm", bufs=3)
                    for kk in range(8):
                        for half in range(2):
                            nc.tensor.matmul(acc[:, half, :], w_sb[:, kk, :],
                                             sts[kk][:, 512 * half:512 * (half + 1)],
                                             start=(kk == 0), stop=(kk == 7))
                    nc.scalar.activation(out=out_ap, in_=acc,
                                         func=Tanh, bias=b_sb)
                # transpose each 128-col block of vt into v5 (both heads at once)
                for j in range(8):
                    t_idx = 8 * tt + j
                    trp = psum.tile([128, 1024], bf16, tag="acc", bufs=2)
                    nc.tensor.transpose(trp[:, 0:128],
                                        vt_c[:, 128 * j:128 * (j + 1)], ident_sb)
                    nc.vector.tensor_copy(
                        v5_sb.rearrange("p t (a b) -> p t a b", a=2)[:, t_idx, :, 0:DH],
                        trp[:, 0:128].rearrange("p (a b) -> p a b", a=2),
                    )

            # ---- phase 2: causal attention, h-outer for split A2A ----
            # The inner loop is software-pipelined: score matmuls run two
            # groups ahead of the context matmuls so the PE never waits on
            # the exp, and each qi's normalize/flush chain is deferred into
            # the next qi's score stream so the PE broadcast matmul never
            # waits on the DVE reciprocal.
            for h in range(HPC):
                hp = DH * h

                def flush_group(grp):
                    # batched 1/l: copy each group's l-row to a distinct
                    # 32-aligned partition, one reciprocal serves them all
                    lb = cxp.tile([64, 512], f32, tag="lb", bufs=1)
                    for i, (cl_sb, _) in enumerate(grp):
                        nc.vector.tensor_copy(lb[32 * i:32 * i + 1, :],
                                              cl_sb[DH:DH + 1, :])
                    rbf = cxp.tile([64, 512], f32, tag="rbf", bufs=1)
                    nc.vector.reciprocal_approx_fast(out=rbf, in_=lb)
                    rbat = cxp.tile([64, 512], bf16, tag="rbat", bufs=1)
                    nc.vector.tensor_copy(rbat, rbf)
                    for i, (cl_sb, tb_idx) in enumerate(grp):
                        rbp = psum.tile([128, 512], f32, tag="acc", bufs=2)
                        nc.tensor.matmul(rbp[0:DH, :], ones_sb[32 * i:32 * i + 1, :],
                                         rbat[32 * i:32 * i + 1, :],
                                         start=True, stop=True,
                                         tile_position=(32 * i, 0))
                        cx = cxp.tile([DH, 512], bf16, tag="cx")
                        nc.vector.tensor_mul(cx, cl_sb[0:DH, :], rbp[0:DH, :])
                        nc.sync.dma_start(out=a2a_in[h][tb_idx, :, :], in_=cx)

                deferred = None  # previous qi's blocks awaiting normalize
                for qi in range(4):
                    nkt = 4 * qi + 4       # causal kt tiles (128 wide)
                    q_lo = 512 * qi
                    ctxps = [psum.tile([128, 512], f32, tag="acc", bufs=2,
                                       name=f"ctxp_h{h}q{qi}b{b}")
                             for b in range(B)]

                    def issue_ctx(ch, b, et):
                        for j in range(2):
                            ktj = 2 * ch + j
                            m = ktj - 4 * qi
                            lo = 128 * m if m > 0 else 0
                            nc.tensor.matmul(
                                ctxps[b][0:DH + 1, lo:512],
                                v5_sb[:, 16 * b + ktj, 65 * h:65 * h + 65],
                                et[:, j, lo:512],
                                start=(ktj == 0), stop=(ktj == nkt - 1),
                            )

                    # supergroups of <=3: issue the chunk's scores
                    # back-to-back, then its ctx pairs — the exp runs a few
                    # groups deep so the PE stream rarely waits on the ACT
                    gseq = [(c, b) for c in range(nkt // 2) for b in range(B)]
                    nch = (len(gseq) + 2) // 3
                    bounds = [len(gseq) * i // nch for i in range(nch + 1)]
                    for ci, cj in zip(bounds[:-1], bounds[1:]):
                        ets = []
                        for ch, b in gseq[ci:cj]:
                            stp = psum.tile([128, 2, 512], f32, tag="mm", bufs=3)
                            for j in range(2):
                                ktj = 2 * ch + j
                                k_lo = 128 * ktj
                                m = ktj - 4 * qi  # >=0: diagonal key tile
                                lo = 128 * m if m > 0 else 0
                                nc.tensor.matmul(
                                    stp[:, j, lo:512],
                                    kt_sb[hp:hp + DH,
                                          2048 * b + k_lo:2048 * b + k_lo + 128],
                                    qt_sb[hp:hp + DH,
                                          2048 * b + q_lo + lo:2048 * b + q_lo + 512],
                                    start=True, stop=m < 0,
                                )
                                if m >= 0:  # causal bias via PE accumulate
                                    nc.tensor.matmul(
                                        stp[:, j, lo:lo + 128], ident_sb,
                                        tri_sb,
                                        start=False, stop=True,
                                    )
                            m0 = 2 * ch - 4 * qi  # j=0 diagonal index
                            elo = 128 * m0 if m0 > 0 else 0
                            et = etp.tile([128, 2, 512], bf16, tag="et")
                            nc.scalar.activation(out=et[:, :, elo:512],
                                                 in_=stp[:, :, elo:512],
                                                 func=Exp, scale=0.125)
                            ets.append((ch, b, et))
                        for ch, b, et in ets:
                            issue_ctx(ch, b, et)
                        if ci == 0 and deferred is not None:
                            flush_group(deferred)
                            deferred = None

                    group = []
                    for b in range(B):
                        # copy ctx+l out of PSUM eagerly (PSUM slot recycles)
                        cl_sb = cxp.tile([DH + 1, 512], f32, tag="cl", bufs=5)
                        nc.vector.tensor_copy(cl_sb, ctxps[b][0:DH + 1, :])
                        group.append((cl_sb, 4 * b + qi))
                    if qi == 3:            # last qi gates the A2A: no defer
                        flush_group(group)
                    else:
                        deferred = group
                # per-head exchange: h=0 overlaps h=1 compute
                nc.gpsimd.collective_compute(
                    "AllToAll", mybir.AluOpType.bypass,
                    replica_groups=[list(range(N_CORES))],
                    ins=[a2a_in[h][:].opt()], outs=[a2a_out[h][:].opt()],
                )

            # ---- phase 3: output projection, per-half accumulation ----
            nc.gpsimd.dma_start(out=wo_sb,
                                in_=wo.ap().rearrange("(k p) o -> p k o", p=128))
            nc.gpsimd.dma_start(out=bo_sb,
                                in_=bo.ap().rearrange("(k p) one -> p k one", p=128))
            cxt0s, cxt1s = [], []
            for kc in range(8):
                cxt0 = outp.tile([128, 512], bf16, tag="cxt0", bufs=8)
                cxt1 = outp.tile([128, 512], bf16, tag="cxt1", bufs=8)
                eng0 = nc.sync if kc % 2 == 0 else nc.gpsimd
                eng1 = nc.gpsimd if kc % 2 == 0 else nc.sync
                eng0.dma_start(out=cxt0[0:DH, :], in_=a2a_out[0][kc, :, :])
                eng1.dma_start(out=cxt1[DH:128, :], in_=a2a_out[1][kc, :, :])
                cxt0s.append(cxt0)
                cxt1s.append(cxt1)
            s0s = []
            for oc in range(8):  # h=0 half: runs as soon as A2A#1 lands
                op0 = psum.tile([128, 512], f32, tag="acc", bufs=2)
                osl = slice(128 * oc, 128 * (oc + 1))
                for kc in range(8):
                    nc.tensor.matmul(op0, wo_sb[0:DH, kc, osl], cxt0s[kc][0:DH, :],
                                     start=(kc == 0), stop=(kc == 7))
                s0 = outp.tile([128, 512], f32, tag="s0", bufs=8)
                nc.vector.tensor_copy(s0, op0)
                s0s.append(s0)
            # governor warm-up: dependency-free matmuls that execute inside
            # the h=1 exchange window (otherwise pure PE idle) so the h=1
            # projection below starts at the ramped clock
            fp = psum.tile([128, 2, 512], f32, tag="mm", bufs=3)
            for fi in range(28):
                nc.tensor.matmul(fp[:, fi % 2, :], ident_sb,
                                 qt_sb[:, 512 * (fi % 4):512 * (fi % 4) + 512],
                                 start=True, stop=True)
            for oc in range(8):  # h=1 half after A2A#2, then combine
                op1 = psum.tile([128, 512], f32, tag="acc", bufs=2)
                osl = slice(128 * oc, 128 * (oc + 1))
                for kc in range(8):
                    nc.tensor.matmul(op1, wo_sb[DH:128, kc, osl],
                                     cxt1s[kc][DH:128, :],
                                     start=(kc == 0), stop=(kc == 7))
                s1 = outp.tile([128, 512], f32, tag="s1", bufs=2)
                nc.vector.tensor_add(s1, s0s[oc], op1)
                osb = outp.tile([128, 512], f32, tag="osb", bufs=2)
                nc.scalar.activation(out=osb, in_=s1, func=Tanh, bias=bo_sb[:, oc, :])
                nc.sync.dma_start(out=out[osl, :], in_=osb)

    nc.compile()
    return nc


def _get_nc():
    if "nc" not in _CACHE:
        _CACHE["nc"] = _build()
    return _CACHE["nc"]


def kernel(states, Wq, bq, Wk, bk, Wv, bv, Wo, bo):
    global LAST_RESULTS
    import ml_dtypes
    bf = ml_dtypes.bfloat16
    states = np.asarray(states, dtype=np.float32)
    Wq, Wk, Wv, Wo = (np.asarray(w, dtype=np.float32) for w in (Wq, Wk, Wv, Wo))
    bq, bk, bv, bo = (np.asarray(x, dtype=np.float32) for x in (bq, bk, bv, bo))

    statesT = np.ascontiguousarray(states.reshape(T, D).T).astype(bf)
    # tri[k, c] = NEG where query column c (within the diagonal 128-block)
    # is strictly left of key row k
    k_idx = np.arange(128)[:, None]
    c_idx = np.arange(128)[None, :]
    tri = np.where(c_idx >= k_idx, 0.0, NEG).astype(bf)
    ident16 = np.eye(128, dtype=np.float32).astype(bf)
    ones = np.ones((128, 64), dtype=np.float32).astype(bf)

    in_maps = []
    for c in range(N_CORES):
        sl = slice(LC * c, LC * (c + 1))
        in_maps.append({
            "statesT": statesT,
            "wq": np.ascontiguousarray(Wq[:, sl]).astype(bf),
            "wk": np.ascontiguousarray(Wk[:, sl]).astype(bf),
            "wv": np.ascontiguousarray(Wv[:, sl]).astype(bf),
            "wo": Wo.astype(bf),
            "bq": np.ascontiguousarray(bq[sl]).reshape(LC, 1),
            "bk": np.ascontiguousarray(bk[sl]).reshape(LC, 1),
            "bv": np.ascontiguousarray(bv[sl]).reshape(LC, 1),
            "bo": bo.reshape(D, 1),
            "tri": tri,
            "ident16": ident16,
            "ones": ones,
        })

    nc = _get_nc()
    res = run_bass_kernel_spmd(nc, in_maps, core_ids=list(range(N_CORES)))
    LAST_RESULTS = res

    full = np.empty((T, D), dtype=np.float32)
    for c in range(N_CORES):
        full[TBLK * c:TBLK * (c + 1), :] = res.results[c]["out"].T
    return full.reshape(B, S, D)


# revision 26
# speedup vs baseline: 1.0732x; 1.0666x over previous
"""Trainium2 Bass kernel for nn_AttentionLayer (B=2, S=2048, D=1024, H=16, dh=64).

Sharding: head-parallel across 8 NeuronCores — each core computes the Q/K/V
projections for its 2 heads (column slices of Wq/Wk/Wv), causal attention for
its 4 (batch, head) pairs, then an AllToAll exchanges per-head context so each
core runs the output projection for 1/8 of the tokens.

All matmuls run in bf16 (full PE rate + fast weight load); PSUM accumulation
stays fp32. Softmax skips the max-subtraction (|scores| <= 8 after the
1/sqrt(64) scale, since q/k are tanh outputs), so exp is a single ACT pass and
row sums come from an appended ones-column in the alpha @ V matmul.

Causal structure is exploited at 128-column granularity: for diagonal key
tiles the score/context matmuls cover only the valid query columns, and the
mask add is a single 128-wide triangular matmul.

The AllToAll is split per local head: the h=0 exchange overlaps the h=1
attention compute, and the output projection accumulates each 64-row half as
soon as its exchange lands (K=64 row-packed matmuls).

Self-contained: accepts the full unsharded inputs, returns the full output.
"""

import os

import numpy as np

import concourse.bass as bass
import concourse.mybir as mybir
import concourse.tile as tile
from concourse import bacc
from concourse.bass_utils import run_bass_kernel_spmd

B, S, D = 2, 2048, 1024
H, DH = 16, 64
N_CORES = 8
HPC = H // N_CORES          # heads per core (2)
LC = HPC * DH               # local projection columns (128)
T = B * S                   # total tokens (4096)
TBLK = T // N_CORES         # tokens per output block (512)
NEG = -1.0e9

f32 = mybir.dt.float32
f32r = mybir.dt.float32r
bf16 = mybir.dt.bfloat16

_CACHE = {}
LAST_RESULTS = None


def _build():
    nc = bacc.Bacc("TRN2", target_bir_lowering=False, debug=False,
                   num_devices=N_CORES)

    statesT = nc.dram_tensor("statesT", [D, T], bf16, kind="ExternalInput")
    wq = nc.dram_tensor("wq", [D, LC], bf16, kind="ExternalInput")
    wk = nc.dram_tensor("wk", [D, LC], bf16, kind="ExternalInput")
    wv = nc.dram_tensor("wv", [D, LC], bf16, kind="ExternalInput")
    wo = nc.dram_tensor("wo", [D, D], bf16, kind="ExternalInput")
    bq = nc.dram_tensor("bq", [LC, 1], f32, kind="ExternalInput")
    bk = nc.dram_tensor("bk", [LC, 1], f32, kind="ExternalInput")
    bv = nc.dram_tensor("bv", [LC, 1], f32, kind="ExternalInput")
    bo = nc.dram_tensor("bo", [D, 1], f32, kind="ExternalInput")
    tri01 = nc.dram_tensor("tri01", [128, 128], bf16, kind="ExternalInput")
    ident16 = nc.dram_tensor("ident16", [128, 128], bf16, kind="ExternalInput")
    ones = nc.dram_tensor("ones", [128, 64], bf16, kind="ExternalInput")

    a2a_in = [nc.dram_tensor(f"a2a_in{h}", [N_CORES, DH, TBLK], bf16)
              for h in range(HPC)]
    a2a_out = [nc.dram_tensor(f"a2a_out{h}", [N_CORES, DH, TBLK], bf16)
               for h in range(HPC)]
    out = nc.dram_tensor("out", [D, TBLK], f32, kind="ExternalOutput")

    Tanh = mybir.ActivationFunctionType.Tanh
    Exp = mybir.ActivationFunctionType.Exp

    with tile.TileContext(nc) as tc:
        with (
            tc.tile_pool(name="consts", bufs=1) as consts,
            tc.tile_pool(name="persist", bufs=1) as persist,
            tc.tile_pool(name="stream", bufs=4) as stream,
            tc.tile_pool(name="vtp", bufs=2) as vtp,
            tc.tile_pool(name="etp", bufs=7) as etp,
            tc.tile_pool(name="cxp", bufs=2) as cxp,
            tc.tile_pool(name="outp", bufs=3) as outp,
            # one PSUM pool for the whole program: tag "mm" [128,2,512] x3 =
            # 6 banks, tag "acc" [128,512] x2 = 2 banks -> 8 banks total
            tc.tile_pool(name="psum", bufs=1, space="PSUM") as psum,
        ):
            # ---- constants / weights in SBUF ----
            # DMA order is startup-critical: biases + small consts first,
            # then weight slices interleaved with the first statesT tiles so
            # the first matmul can start as early as possible.
            bq_sb = consts.tile([LC, 1], f32)
            bk_sb = consts.tile([LC, 1], f32)
            bv_sb = consts.tile([LC, 1], f32)
            nc.sync.dma_start(out=bq_sb, in_=bq[:, :])
            nc.sync.dma_start(out=bk_sb, in_=bk[:, :])
            nc.sync.dma_start(out=bv_sb, in_=bv[:, :])
            ident_sb = consts.tile([128, 128], bf16)
            tri_sb = consts.tile([128, 128], bf16)
            ones_sb = consts.tile([128, 64], bf16)

            wq_sb = consts.tile([128, 8, LC], bf16)
            wk_sb = consts.tile([128, 8, LC], bf16)
            wv_sb = consts.tile([128, 8, LC], bf16)
            wq_ap = wq.ap().rearrange("(k p) l -> p k l", p=128)
            wk_ap = wk.ap().rearrange("(k p) l -> p k l", p=128)
            wv_ap = wv.ap().rearrange("(k p) l -> p k l", p=128)
            # wo/bo are only needed by phase 3; keep them on the gpsimd
            # queue behind the statesT tiles it also carries.
            wo_sb = persist.tile([128, 8, D], bf16)
            bo_sb = consts.tile([128, 8, 1], f32)

            # ---- phase 1: Q/K/V projections (transposed layout) ----
            # h-major tiles: partitions = (head*64 + d), free = (batch*S + s).
            # One tanh per projection per token tile, and the two batches'
            # score matmuls row-pack into different PE row groups via the
            # stationary partition offset.
            qt_sb = persist.tile([128, T], bf16, name="qt")
            kt_sb = persist.tile([128, T], bf16, name="kt")
            # v5: per 128-token tile, [tok_local, (h0 V | ones | h1 V | ones)]
            v5_sb = persist.tile([128, T // 128, 2 * (DH + 1)], bf16)

            for tt in range(T // 1024):  # 4 double-width token tiles
                # three passes (q, k, v) over prefetched statesT tiles: each
                # pass's PSUM slot was drained by a tanh two passes ago, so
                # the PE never waits at token-tile boundaries
                sts = []
                for kk in range(8):
                    if tt == 0:
                        nc.sync.dma_start(out=wq_sb[:, kk, :], in_=wq_ap[:, kk, :])
                        nc.sync.dma_start(out=wk_sb[:, kk, :], in_=wk_ap[:, kk, :])
                        nc.sync.dma_start(out=wv_sb[:, kk, :], in_=wv_ap[:, kk, :])
                    st = stream.tile([128, 1024], bf16, tag="st", bufs=12)
                    dma_eng = nc.sync if (tt < 2 or kk % 2 == 0) else nc.gpsimd
                    dma_eng.dma_start(
                        out=st,
                        in_=statesT[128 * kk:128 * (kk + 1),
                                    1024 * tt:1024 * (tt + 1)],
                    )
                    sts.append(st)
                    if tt == 0 and kk == 0:
                        # small consts off the first-matmul critical DMA path
                        nc.sync.dma_start(out=ident_sb, in_=ident16[:, :])
                        nc.sync.dma_start(out=tri_sb, in_=tri01[:, :])
                        nc.sync.dma_start(out=ones_sb, in_=ones[:, :])
                        nc.vector.tensor_copy(
                            v5_sb.rearrange("p t (a b) -> p (t a) b", a=2)[:, :, DH:DH + 1].opt(),
                            ones_sb[:, :].opt(),
                        )
                bb = tt // 2                     # which batch this tt is in
                sl = slice(2048 * bb + 1024 * (tt % 2),
                           2048 * bb + 1024 * (tt % 2) + 1024)
                vt_c = vtp.tile([128, 1024], bf16, tag="vt")
                for w_sb, b_sb, out_ap in (
                        (wq_sb, bq_sb, qt_sb[:, sl]),
                        (wk_sb, bk_sb, kt_sb[:, sl]),
                        (wv_sb, bv_sb, vt_c[:, :])):
                    acc = psum.tile([128, 2, 512], f32, tag="mm", bufs=3)
                    for kk in range(8):
                        for half in range(2):
                            nc.tensor.matmul(acc[:, half, :], w_sb[:, kk, :],
                                             sts[kk][:, 512 * half:512 * (half + 1)],
                                             start=(kk == 0), stop=(kk == 7))
                    nc.scalar.activation(out=out_ap, in_=acc,
                                         func=Tanh, bias=b_sb)
                # transpose each 128-col block of vt into v5 (both heads at once)
                for j in range(8):
                    t_idx = 8 * tt + j
                    trp = psum.tile([128, 1024], bf16, tag="acc", bufs=2)
                    nc.tensor.transpose(trp[:, 0:128],
                                        vt_c[:, 128 * j:128 * (j + 1)], ident_sb)
                    nc.vector.tensor_copy(
                        v5_sb.rearrange("p t (a b) -> p t a b", a=2)[:, t_idx, :, 0:DH],
                        trp[:, 0:128].rearrange("p (a b) -> p a b", a=2),
                    )

            # ---- phase 2: causal attention, h-outer for split A2A ----
            # The inner loop is software-pipelined: score matmuls run two
            # groups ahead of the context matmuls so the PE never waits on
            # the exp, and each qi's normalize/flush chain is deferred into
            # the next qi's score stream so the PE broadcast matmul never
            # waits on the DVE reciprocal.
            for h in range(HPC):
                hp = DH * h

                def flush_group(grp):
                    # batched 1/l: copy each group's l-row to a distinct
                    # 32-aligned partition, one reciprocal serves them all
                    lb = cxp.tile([64, 512], f32, tag="lb", bufs=1)
                    for i, (cl_sb, _) in enumerate(grp):
                        nc.vector.tensor_copy(lb[32 * i:32 * i + 1, :],
                                              cl_sb[DH:DH + 1, :])
                    rbf = cxp.tile([64, 512], f32, tag="rbf", bufs=1)
                    nc.vector.reciprocal_approx_fast(out=rbf, in_=lb)
                    rbat = cxp.tile([64, 512], bf16, tag="rbat", bufs=1)
                    nc.vector.tensor_copy(rbat, rbf)
                    for i, (cl_sb, tb_idx) in enumerate(grp):
                        rbp = psum.tile([128, 512], f32, tag="acc", bufs=2)
                        nc.tensor.matmul(rbp[0:DH, :], ones_sb[32 * i:32 * i + 1, :],
                                         rbat[32 * i:32 * i + 1, :],
                                         start=True, stop=True,
                                         tile_position=(32 * i, 0))
                        cx = cxp.tile([DH, 512], bf16, tag="cx")
                        nc.vector.tensor_mul(cx, cl_sb[0:DH, :], rbp[0:DH, :])
                        nc.sync.dma_start(out=a2a_in[h][tb_idx, :, :], in_=cx)

                deferred = None  # previous qi's blocks awaiting normalize
                for qi in range(4):
                    nkt = 4 * qi + 4       # causal kt tiles (128 wide)
                    q_lo = 512 * qi
                    ctxps = [psum.tile([128, 512], f32, tag="acc", bufs=2,
                                       name=f"ctxp_h{h}q{qi}b{b}")
                             for b in range(B)]

                    def issue_ctx(ch, b, et):
                        for j in range(2):
                            ktj = 2 * ch + j
                            m = ktj - 4 * qi
                            lo = 128 * m if m > 0 else 0
                            nc.tensor.matmul(
                                ctxps[b][0:DH + 1, lo:512],
                                v5_sb[:, 16 * b + ktj, 65 * h:65 * h + 65],
                                et[:, j, lo:512],
                                start=(ktj == 0), stop=(ktj == nkt - 1),
                            )

                    # supergroups of 3: issue 3 groups' scores back-to-back,
                    # then their 3 ctx pairs — the exp runs 3 groups deep so
                    # the PE stream never waits on the ACT engine
                    gseq = [(c, b) for c in range(nkt // 2) for b in range(B)]
                    for ci in range(0, len(gseq), 3):
                        ets = []
                        for ch, b in gseq[ci:ci + 3]:
                            stp = psum.tile([128, 2, 512], f32, tag="mm", bufs=3)
                            for j in range(2):
                                ktj = 2 * ch + j
                                k_lo = 128 * ktj
                                m = ktj - 4 * qi  # >=0: diagonal key tile
                                lo = 128 * m if m > 0 else 0
                                nc.tensor.matmul(
                                    stp[:, j, lo:512],
                                    kt_sb[hp:hp + DH,
                                          2048 * b + k_lo:2048 * b + k_lo + 128],
                                    qt_sb[hp:hp + DH,
                                          2048 * b + q_lo + lo:2048 * b + q_lo + 512],
                                    start=True, stop=True,
                                )
                            m0 = 2 * ch - 4 * qi  # j=0 diagonal index
                            elo = 128 * m0 if m0 > 0 else 0
                            et = etp.tile([128, 2, 512], bf16, tag="et")
                            nc.scalar.activation(out=et[:, :, elo:512],
                                                 in_=stp[:, :, elo:512],
                                                 func=Exp, scale=0.125)
                            for j in range(2):  # multiplicative causal mask
                                m = 2 * ch + j - 4 * qi
                                if m >= 0:  # 0/1 triangle on the DVE, off PE
                                    lo = 128 * m
                                    nc.vector.tensor_mul(
                                        et[:, j, lo:lo + 128],
                                        et[:, j, lo:lo + 128], tri_sb)
                            ets.append((ch, b, et))
                        for ch, b, et in ets:
                            issue_ctx(ch, b, et)
                        if ci == 0 and deferred is not None:
                            flush_group(deferred)
                            deferred = None

                    group = []
                    for b in range(B):
                        # copy ctx+l out of PSUM eagerly (PSUM slot recycles)
                        cl_sb = cxp.tile([DH + 1, 512], f32, tag="cl", bufs=5)
                        nc.vector.tensor_copy(cl_sb, ctxps[b][0:DH + 1, :])
                        group.append((cl_sb, 4 * b + qi))
                    if qi == 3:            # last qi gates the A2A: no defer
                        flush_group(group)
                    else:
                        deferred = group
                # per-head exchange: h=0 overlaps h=1 compute
                nc.gpsimd.collective_compute(
                    "AllToAll", mybir.AluOpType.bypass,
                    replica_groups=[list(range(N_CORES))],
                    ins=[a2a_in[h][:].opt()], outs=[a2a_out[h][:].opt()],
                )

            # ---- phase 3: output projection, per-half accumulation ----
            nc.gpsimd.dma_start(out=wo_sb,
                                in_=wo.ap().rearrange("(k p) o -> p k o", p=128))
            nc.gpsimd.dma_start(out=bo_sb,
                                in_=bo.ap().rearrange("(k p) one -> p k one", p=128))
            cxt0s, cxt1s = [], []
            for kc in range(8):
                cxt0 = outp.tile([128, 512], bf16, tag="cxt0", bufs=8)
                cxt1 = outp.tile([128, 512], bf16, tag="cxt1", bufs=8)
                eng0 = nc.sync if kc % 2 == 0 else nc.gpsimd
                eng1 = nc.gpsimd if kc % 2 == 0 else nc.sync
                eng0.dma_start(out=cxt0[0:DH, :], in_=a2a_out[0][kc, :, :])
                eng1.dma_start(out=cxt1[DH:128, :], in_=a2a_out[1][kc, :, :])
                cxt0s.append(cxt0)
                cxt1s.append(cxt1)
            s0s = []
            for oc in range(8):  # h=0 half: runs as soon as A2A#1 lands
                op0 = psum.tile([128, 512], f32, tag="acc", bufs=2)
                osl = slice(128 * oc, 128 * (oc + 1))
                for kc in range(8):
                    nc.tensor.matmul(op0, wo_sb[0:DH, kc, osl], cxt0s[kc][0:DH, :],
                                     start=(kc == 0), stop=(kc == 7))
                s0 = outp.tile([128, 512], f32, tag="s0", bufs=8)
                nc.vector.tensor_copy(s0, op0)
                s0s.append(s0)
            for oc in range(8):  # h=1 half after A2A#2, then combine
                op1 = psum.tile([128, 512], f32, tag="acc", bufs=2)
                osl = slice(128 * oc, 128 * (oc + 1))
                for kc in range(8):
                    nc.tensor.matmul(op1, wo_sb[DH:128, kc, osl],
                                     cxt1s[kc][DH:128, :],
                                     start=(kc == 0), stop=(kc == 7))
                s1 = outp.tile([128, 512], f32, tag="s1", bufs=2)
                nc.vector.tensor_add(s1, s0s[oc], op1)
                osb = outp.tile([128, 512], f32, tag="osb", bufs=2)
                nc.scalar.activation(out=osb, in_=s1, func=Tanh, bias=bo_sb[:, oc, :])
                nc.sync.dma_start(out=out[osl, :], in_=osb)

    nc.compile()
    return nc


def _get_nc():
    if "nc" not in _CACHE:
        _CACHE["nc"] = _build()
    return _CACHE["nc"]


def kernel(states, Wq, bq, Wk, bk, Wv, bv, Wo, bo):
    global LAST_RESULTS
    import ml_dtypes
    bf = ml_dtypes.bfloat16
    states = np.asarray(states, dtype=np.float32)
    Wq, Wk, Wv, Wo = (np.asarray(w, dtype=np.float32) for w in (Wq, Wk, Wv, Wo))
    bq, bk, bv, bo = (np.asarray(x, dtype=np.float32) for x in (bq, bk, bv, bo))

    statesT = np.ascontiguousarray(states.reshape(T, D).T).astype(bf)
    # tri01[k, c] = 0 where query column c (within the diagonal 128-block)
    # is strictly left of key row k, else 1 — multiplicative causal mask
    k_idx = np.arange(128)[:, None]
    c_idx = np.arange(128)[None, :]
    tri01 = np.where(c_idx >= k_idx, 1.0, 0.0).astype(bf)
    ident16 = np.eye(128, dtype=np.float32).astype(bf)
    ones = np.ones((128, 64), dtype=np.float32).astype(bf)

    in_maps = []
    for c in range(N_CORES):
        sl = slice(LC * c, LC * (c + 1))
        in_maps.append({
            "statesT": statesT,
            "wq": np.ascontiguousarray(Wq[:, sl]).astype(bf),
            "wk": np.ascontiguousarray(Wk[:, sl]).astype(bf),
            "wv": np.ascontiguousarray(Wv[:, sl]).astype(bf),
            "wo": Wo.astype(bf),
            "bq": np.ascontiguousarray(bq[sl]).reshape(LC, 1),
            "bk": np.ascontiguousarray(bk[sl]).reshape(LC, 1),
            "bv": np.ascontiguousarray(bv[sl]).reshape(LC, 1),
            "bo": bo.reshape(D, 1),
            "tri01": tri01,
            "ident16": ident16,
            "ones": ones,
        })

    nc = _get_nc()
    res = run_bass_kernel_spmd(nc, in_maps, core_ids=list(range(N_CORES)))
    LAST_RESULTS = res

    full = np.empty((T, D), dtype=np.float32)
    for c in range(N_CORES):
        full[TBLK * c:TBLK * (c + 1), :] = res.results[c]["out"].T
    return full.reshape(B, S, D)
